# revision 21
# baseline (speedup 1.0000x reference)
"""Trainium2 Bass kernel for LowRankTriLinearFusionAttn.

Math (per sample b):
  g  = relu(LN(h_g  @ Wg.T + bg))          (256)
  d2 = relu(LN(h_2d @ W2.T + b2))          (256)
  d3 = relu(LN(h_3d @ W3.T + b3))          (256)
  z_r[b,r,:] = (g U_r^T) * (d2 V_r^T) * (d3 S_r^T)     r in 0..15
  beta = softmax(relu([h_g|h_2d|h_3d] @ Wa1.T + ba1) @ Wa2.T + ba2)
  z[b,:] = sum_r beta[b,r] * z_r[b,r,:]

Sharding: pure data parallel over 8 NeuronCores (batch 8192 -> 1024/core).
Host pre-packs weights transposed/bf16 AND the activations transposed
(features-major, bf16) so no x-transposes are needed on the PE.

Engine split in the rank phase (the 60%-of-kernel steady state):
  PE   : rank-expansion matmuls (48 x N=512 per b-tile)
  ACT  : beta-scaled PSUM evictions (per-partition scale = beta[:, r])
  DVE  : first trilinear multiply + bf16 accumulator adds (2x mode)
  GpSimd: second trilinear multiply
"""

import sys
import types

import numpy as np
import ml_dtypes

import concourse.bass as bass
import concourse.tile as tile
from concourse import bacc
from concourse import mybir
from concourse.bass import ts
from concourse.bass_utils import run_bass_kernel_spmd
import bass_rust


def _ensure_ntff_hook():
    """Provide antenv.axon_hooks if the image's antenv stub lacks it, so
    run_bass_kernel_spmd(trace=True) can capture NTFF profiles under axon."""
    try:
        import antenv.axon_hooks  # noqa: F401
        return
    except ImportError:
        pass
    try:
        from trn_agent_boot.trn_boot import _ntff_profile_via_ctypes

        hook = _ntff_profile_via_ctypes("/opt/axon/libaxon_pjrt.so")
    except Exception:
        hook = None
    mod = types.ModuleType("antenv.axon_hooks")
    _state = {"hook": hook}
    mod.get_axon_ntff_profile_hook = lambda: _state["hook"]
    mod.set_axon_ntff_profile_hook = lambda h: _state.update(hook=h)
    sys.modules["antenv.axon_hooks"] = mod


_ensure_ntff_hook()

BF16 = mybir.dt.bfloat16
F32 = mybir.dt.float32
AF = mybir.ActivationFunctionType
OP = mybir.AluOpType

N_CORES = 8
B = 8192
D_G, D_2D, D_3D = 512, 768, 1024
D_CAT = D_G + D_2D + D_3D  # 2304
D_F, RANK, ATTN_H = 256, 16, 512
RD = RANK * D_F  # 4096
P = 128

BC = B // N_CORES           # 1024 samples per core
NBT = BC // P               # 8 batch tiles per core
KD = [D_G // P, D_2D // P, D_3D // P]   # k-tiles per modality: 4, 6, 8
KOFF = [0, KD[0], KD[0] + KD[1]]        # xt chunk offsets: 0, 4, 10
NK = D_CAT // P             # 18
NH = ATTN_H // P            # 4
NCH = RD // 512             # 8 chunks of 512 in the rank-expanded dim
KF = D_F // P               # 2 k-tiles for the 256-dim contraction
EPS = 1e-5
N_WARM = 16                 # PE warm-up transposes before real work


def build_kernel(bc=BC, general_ln=False, use_bias=False, use_ba2=False):
    assert bc % 512 == 0
    nbt = bc // P
    nc = bacc.Bacc("TRN2", debug=False)

    # ---- external I/O (per-core shapes) ----
    xt_in = nc.dram_tensor("xt", [D_CAT, bc], BF16, kind="ExternalInput").ap()
    wc_t = nc.dram_tensor("wc_t", [D_CAT, D_F], BF16, kind="ExternalInput").ap()
    uvs_t = nc.dram_tensor("uvs_t", [6 * P, RD], BF16, kind="ExternalInput").ap()
    wa1_t = nc.dram_tensor("wa1_t", [D_CAT, ATTN_H], BF16, kind="ExternalInput").ap()
    wa2_t = nc.dram_tensor("wa2_t", [ATTN_H, RANK], BF16, kind="ExternalInput").ap()
    consts_f = nc.dram_tensor("consts_f", [P, 16], F32, kind="ExternalInput").ap()
    consts_b = nc.dram_tensor("consts_b", [4, D_F], BF16, kind="ExternalInput").ap()
    ident_in = nc.dram_tensor("ident", [P, P], BF16, kind="ExternalInput").ap()
    z_out = nc.dram_tensor("z", [bc, D_F], F32, kind="ExternalOutput").ap()

    from contextlib import ExitStack

    with tile.TileContext(nc) as tc, ExitStack() as ctx:
        consts = ctx.enter_context(tc.tile_pool(name="consts", bufs=1))
        wpool = ctx.enter_context(tc.tile_pool(name="w", bufs=1))
        xtp = ctx.enter_context(tc.tile_pool(name="xt", bufs=1))
        gtp = ctx.enter_context(tc.tile_pool(name="gt", bufs=1))
        sp = ctx.enter_context(tc.tile_pool(name="sp", bufs=16))
        zp = ctx.enter_context(tc.tile_pool(name="zacc", bufs=4))
        cp = ctx.enter_context(tc.tile_pool(name="cp", bufs=8))
        pp = ctx.enter_context(tc.tile_pool(name="ps", bufs=8, space="PSUM"))

        # ---------- constants / small weights (scalar HWDGE queue) ----------
        # wc per modality so the first projections gate on a small DMA
        wc_sb = wpool.tile([P, NK, D_F], BF16, tag="wc")
        wc_r = wc_t.rearrange("(t p) n -> p t n", p=P)
        for m in range(3):
            nc.scalar.dma_start(
                out=wc_sb[:, KOFF[m] : KOFF[m] + KD[m], :],
                in_=wc_r[:, KOFF[m] : KOFF[m] + KD[m], :],
            )
        cf_sb = consts.tile([P, 16], F32, tag="cf")  # ba1(4) lnw(6) lnb(6)
        nc.scalar.dma_start(out=cf_sb, in_=consts_f)
        identity = consts.tile([P, P], BF16, tag="ident")
        nc.scalar.dma_start(out=identity, in_=ident_in)
        ba1_sb = cf_sb[:, 0:NH]
        lnw_sb = cf_sb[:, 4:10]
        lnb_sb = cf_sb[:, 10:16]
        if use_bias or use_ba2:
            cb_sb = consts.tile([1, 4, D_F], BF16, tag="cb")
            nc.scalar.dma_start(
                out=cb_sb, in_=consts_b.rearrange("(o m) n -> o m n", o=1)
            )
            bias_sb = cb_sb[:, 0:3, :]
            ba2_sb = cb_sb[:, 3, 0:RANK]
            ones_row = consts.tile([1, P], BF16, tag="ones")
            nc.vector.memset(ones_row, 1.0)
        eps_t = consts.tile([P, 1], F32, tag="eps")
        nc.vector.memset(eps_t, EPS)
        warm_sb = consts.tile([P, P], BF16, tag="warm")
        nc.vector.memset(warm_sb, 1.0)

        # ---------- transposed activations (sync HWDGE queue, k-order) ----
        # grouped DMAs amortize the ~0.64us per-DMA issue cost; group edges
        # align with the modality boundaries the m-major proj sweep needs
        xt_sb = xtp.tile([P, NK, bc], BF16, tag="xt")
        xt = [xt_sb[:, k, :] for k in range(NK)]
        xt_r = xt_in.rearrange("(t p) n -> p t n", p=P)
        xt_dmas = []
        for a, b in ((0, 2), (2, 4), (4, 7), (7, 10), (10, 14), (14, 18)):
            d = nc.sync.dma_start(out=xt_sb[:, a:b, :], in_=xt_r[:, a:b, :])
            xt_dmas.append(d)

        # ---------- big weights ----------
        # wa1/wa2 on the scalar HWDGE queue (gpsimd SWDGE is descriptor-rate
        # limited: ~2300 descriptors would take ~25us); a probe read on the
        # same queue orders wa1's transfer after the xt k0-9 groups so the
        # activations win the early HBM race. uvs stays on gpsimd, gated on
        # wa1's completion via its own probe.
        probe = consts.tile([1, 2], BF16, tag="probe")
        nc.scalar.copy(probe, xt[9][0:1, 0:2])
        wa1_sb = wpool.tile([P, NK, ATTN_H], BF16, tag="wa1")
        wa1_r = wa1_t.rearrange("(t p) n -> p t n", p=P)
        d_wa1 = nc.scalar.dma_start(out=wa1_sb, in_=wa1_r)
        wa2_sb = wpool.tile([P, NH, RANK], BF16, tag="wa2")
        d_wa2 = nc.scalar.dma_start(
            out=wa2_sb, in_=wa2_t.rearrange("(t p) n -> p t n", p=P)
        )
        probe2 = consts.tile([1, 2], BF16, tag="probe2")
        nc.gpsimd.tensor_copy(probe2, wa1_sb[0:1, 0, 0:2])
        uvs_sb = wpool.tile([P, 6, RD], BF16, tag="uvs")
        uvs_r = uvs_t.rearrange("(t p) n -> p t n", p=P)
        d_uvs = []
        for q in range(4):
            d = nc.gpsimd.dma_start(
                out=uvs_sb[:, :, ts(q, RD // 4)], in_=uvs_r[:, :, ts(q, RD // 4)]
            )
            d_uvs.append(d)

        # ---------- PE warm-up: ramp the clock while DMAs land ----------
        for w in range(N_WARM):
            wps = pp.tile([P, P], BF16, tag="ps", name="warm")
            nc.tensor.transpose(wps, warm_sb, warm_sb)

        # ---------- projections + LN (m-major follows DMA arrival) -------
        ups = [[None] * 3 for _ in range(nbt)]

        def emit_proj(t, m):
            ps = pp.tile([P, D_F], F32, tag="ps", name="ps_proj")
            for k in range(KD[m]):
                nc.tensor.matmul(
                    ps,
                    lhsT=xt[KOFF[m] + k][:, ts(t, P)],
                    rhs=wc_sb[:, KOFF[m] + k, :],
                    start=(k == 0),
                    stop=(k == KD[m] - 1) and not use_bias,
                )
            if use_bias:
                nc.tensor.matmul(
                    ps, lhsT=ones_row, rhs=bias_sb[:, m, :], start=False, stop=True
                )
            stats = sp.tile([P, 6], F32, tag="stats", name="stats")
            nc.vector.bn_stats(stats, ps)
            mv = sp.tile([P, 2], F32, tag="mv", name="mv")
            nc.vector.bn_aggr(mv, stats)
            sd = sp.tile([P, 1], F32, tag="sd", name="sd")
            nc.scalar.activation(sd, mv[:, 1:2], AF.Sqrt, bias=eps_t, scale=1.0)
            rstd = sp.tile([P, 1], F32, tag="rstd", name="rstd")
            nc.vector.reciprocal(rstd, sd)
            u = gtp.tile([P, D_F], BF16, tag=f"u{t}_{m}", name=f"u{t}_{m}")
            nc.vector.tensor_scalar(
                out=u,
                in0=ps,
                scalar1=mv[:, 0:1],
                scalar2=rstd,
                op0=OP.subtract,
                op1=OP.mult,
            )
            ups[t][m] = u

        for m in range(3):
            for t in range(nbt):
                emit_proj(t, m)

        # ---------- attention layer 1 (transposed: h on partitions) ------
        a1t = wpool.tile([P, NH, bc], BF16, tag="a1t")

        def emit_attn_l1(c, h):
            ps = pp.tile([P, 512], F32, tag="ps", name="ps_a1")
            for k in range(NK):
                nc.tensor.matmul(
                    ps,
                    lhsT=wa1_sb[:, k, ts(h, P)],
                    rhs=xt[k][:, ts(c, 512)],
                    start=(k == 0),
                    stop=(k == NK - 1),
                )
            nc.scalar.activation(
                a1t[:, h, ts(c, 512)],
                ps,
                AF.Relu,
                bias=ba1_sb[:, h : h + 1],
                scale=1.0,
            )

        # ---------- per-tile tail stages ----------
        gts = [None] * nbt
        betas = [None] * nbt

        def emit_a2_softmax(t):
            ps = pp.tile([P, RANK], F32, tag="ps", name="ps_a2")
            for k in range(NH):
                nc.tensor.matmul(
                    ps,
                    lhsT=a1t[:, k, ts(t, P)],
                    rhs=wa2_sb[:, k, :],
                    start=(k == 0),
                    stop=(k == NH - 1) and not use_ba2,
                )
            if use_ba2:
                nc.tensor.matmul(ps, lhsT=ones_row, rhs=ba2_sb, start=False, stop=True)
            negm = sp.tile([P, 1], F32, tag="negm", name="negm")
            nc.vector.reduce_max(negm, ps, axis=mybir.AxisListType.X, negate=True)
            e = sp.tile([P, RANK], F32, tag="esm", name="esm")
            ssum = sp.tile([P, 1], F32, tag="ssum", name="ssum")
            nc.scalar.activation(e, ps, AF.Exp, bias=negm, scale=1.0, accum_out=ssum)
            rs = sp.tile([P, 1], F32, tag="rs", name="rs")
            nc.vector.reciprocal(rs, ssum)
            beta = gtp.tile([P, RANK], F32, tag=f"beta{t}", name=f"beta{t}")
            nc.vector.tensor_scalar_mul(beta, e, rs)
            betas[t] = beta

        def emit_gtt(t):
            gt = gtp.tile([P, 6, P], BF16, tag=f"gt{t}", name=f"gt{t}")
            for m in range(3):
                u = ups[t][m]
                tp = pp.tile([P, KF, P], BF16, tag="ps", name="tpg")
                for j in range(KF):
                    nc.tensor.transpose(tp[:, j, :], u[:, ts(j, P)], identity)
                if general_ln:
                    for j in range(KF):
                        col = m * KF + j
                        nc.scalar.activation(
                            gt[:, m * KF + j, :],
                            tp[:, j, :],
                            AF.Relu,
                            bias=lnb_sb[:, col : col + 1],
                            scale=lnw_sb[:, col : col + 1],
                        )
                else:
                    # identity LN affine: relu-only evict (ACT; DVE is the
                    # critical engine in the rank waves)
                    nc.scalar.activation(
                        gt[:, m * KF : (m + 1) * KF, :], tp, AF.Relu,
                        bias=0.0, scale=1.0,
                    )
            gts[t] = gt

        def emit_rank(t):
            beta = betas[t]
            gt = gts[t]
            acc = zp.tile([P, 512], BF16, tag="acc", name="acc")
            for c in range(NCH):
                pz = []
                for m in range(3):
                    ps = pp.tile([P, 512], F32, tag="ps", name="ps_rk")
                    for k in range(KF):
                        nc.tensor.matmul(
                            ps,
                            lhsT=gt[:, m * KF + k, :],
                            rhs=uvs_sb[:, m * KF + k, ts(c, 512)],
                            start=(k == 0),
                            stop=(k == KF - 1),
                        )
                    pz.append(ps)
                # beta folded into the pz0 eviction (per-partition scale, ACT)
                ugb = cp.tile([P, 512], BF16, tag="ugb", name="ugb")
                for rr in range(2):
                    r = 2 * c + rr
                    nc.scalar.activation(
                        ugb[:, ts(rr, D_F)],
                        pz[0][:, ts(rr, D_F)],
                        AF.Copy,
                        scale=beta[:, r : r + 1],
                    )
                tm = cp.tile([P, 512], BF16, tag="tm", name="tm")
                nc.vector.tensor_tensor(tm, ugb, pz[1], op=OP.mult)
                if c == 0:
                    nc.vector.tensor_tensor(acc, tm, pz[2], op=OP.mult)
                else:
                    t2 = cp.tile([P, 512], BF16, tag="t2", name="t2")
                    nc.vector.tensor_tensor(t2, tm, pz[2], op=OP.mult)
                    # bf16 SBUF-only accumulate rides the otherwise-idle GpSimd
                    nc.gpsimd.tensor_tensor(acc, t2, acc, op=OP.add)
            zfin = zp.tile([P, D_F], F32, tag="zfin", name="zfin")
            nc.gpsimd.tensor_tensor(
                zfin, acc[:, 0:D_F], acc[:, D_F : 2 * D_F], op=OP.add
            )
            nc.sync.dma_start(out=z_out[ts(t, P), :], in_=zfin)

        # ---------- wave schedule ----------
        # gtt transposes interleave with attn1 h-groups (each gtt only needs
        # its own LN chain done, not the whole DVE backlog). attn1 chunk-1
        # h-groups are spread between early rank tiles: each is ~4us of PE
        # work with no ACT/DVE load, a catch-up window for the trilinear
        # elementwise backlog; late gtt evicts ride those windows too.
        emit_attn_l1(0, 0)
        emit_gtt(0)
        emit_attn_l1(0, 1)
        emit_gtt(1)
        emit_attn_l1(0, 2)
        emit_gtt(2)
        emit_attn_l1(0, 3)
        for t in range(4):
            emit_a2_softmax(t)
        emit_rank(0)
        emit_attn_l1(1, 0)
        emit_gtt(3)
        emit_rank(1)
        emit_attn_l1(1, 1)
        emit_gtt(4)
        emit_rank(2)
        emit_attn_l1(1, 2)
        emit_gtt(5)
        emit_rank(3)
        emit_attn_l1(1, 3)
        emit_gtt(6)
        emit_gtt(7)
        for t in range(4, nbt):
            emit_a2_softmax(t)
        for t in range(4, nbt):
            emit_rank(t)

    nc.compile()
    return nc


_BF = ml_dtypes.bfloat16


def _pack_weights(inputs):
    """Host-side offline packing: transpose + cast weights once."""
    f = np.asarray
    wc_t = np.concatenate(
        [f(inputs["Wg"]).T, f(inputs["W2"]).T, f(inputs["W3"]).T], axis=0
    ).astype(_BF)  # [2304, 256]
    uvs_t = np.concatenate(
        [f(inputs["U"]).T, f(inputs["V"]).T, f(inputs["S"]).T], axis=0
    ).astype(_BF)  # [768, 4096]
    wa1_t = np.ascontiguousarray(f(inputs["Wa1"]).T).astype(_BF)  # [2304, 512]
    wa2_t = np.ascontiguousarray(f(inputs["Wa2"]).T).astype(_BF)  # [512, 16]
    consts_b = np.zeros((4, D_F), dtype=_BF)
    consts_b[0] = f(inputs["bg"]).astype(_BF)
    consts_b[1] = f(inputs["b2"]).astype(_BF)
    consts_b[2] = f(inputs["b3"]).astype(_BF)
    consts_b[3, :RANK] = f(inputs["ba2"]).astype(_BF)
    consts_f = np.concatenate(
        [
            f(inputs["ba1"]).reshape(NH, P).T,
            np.concatenate(
                [
                    f(inputs["ln_g_w"]).reshape(KF, P),
                    f(inputs["ln_2_w"]).reshape(KF, P),
                    f(inputs["ln_3_w"]).reshape(KF, P),
                ],
                axis=0,
            ).T,
            np.concatenate(
                [
                    f(inputs["ln_g_b"]).reshape(KF, P),
                    f(inputs["ln_2_b"]).reshape(KF, P),
                    f(inputs["ln_3_b"]).reshape(KF, P),
                ],
                axis=0,
            ).T,
        ],
        axis=1,
    ).astype(np.float32)  # [128, 16]
    return {
        "ident": np.eye(P, dtype=_BF),
        "wc_t": wc_t,
        "uvs_t": uvs_t,
        "wa1_t": wa1_t,
        "wa2_t": wa2_t,
        "consts_f": consts_f,
        "consts_b": consts_b,
    }


_NC_CACHE = {}


def _get_nc(general_ln, use_bias, use_ba2):
    key = (general_ln, use_bias, use_ba2)
    if key not in _NC_CACHE:
        _NC_CACHE[key] = build_kernel(
            general_ln=general_ln, use_bias=use_bias, use_ba2=use_ba2
        )
    return _NC_CACHE[key]


def kernel(run_opts=None, **inputs):
    f64 = np.float64
    general_ln = not (
        np.all(np.asarray(inputs["ln_g_w"], f64) == 1.0)
        and np.all(np.asarray(inputs["ln_2_w"], f64) == 1.0)
        and np.all(np.asarray(inputs["ln_3_w"], f64) == 1.0)
        and np.all(np.asarray(inputs["ln_g_b"], f64) == 0.0)
        and np.all(np.asarray(inputs["ln_2_b"], f64) == 0.0)
        and np.all(np.asarray(inputs["ln_3_b"], f64) == 0.0)
    )
    use_bias = not (
        np.all(np.asarray(inputs["bg"], f64) == 0.0)
        and np.all(np.asarray(inputs["b2"], f64) == 0.0)
        and np.all(np.asarray(inputs["b3"], f64) == 0.0)
    )
    use_ba2 = not np.all(np.asarray(inputs["ba2"], f64) == 0.0)
    nc = _get_nc(general_ln, use_bias, use_ba2)
    wmap = _pack_weights(inputs)
    x_all = np.concatenate(
        [
            np.asarray(inputs["h_g"], dtype=np.float32),
            np.asarray(inputs["h_2d"], dtype=np.float32),
            np.asarray(inputs["h_3d"], dtype=np.float32),
        ],
        axis=1,
    )  # [B, 2304]

    in_maps = []
    for i in range(N_CORES):
        sl = slice(i * BC, (i + 1) * BC)
        m = dict(wmap)
        m["xt"] = x_all[sl].T.astype(_BF)  # [2304, 1024] contiguous bf16
        in_maps.append(m)

    res = run_bass_kernel_spmd(
        nc, in_maps, core_ids=list(range(N_CORES)), **(run_opts or {})
    )
    out = np.concatenate([r["z"] for r in res.results], axis=0)
    if run_opts:
        kernel.last_results = res
    return out


# revision 22
# speedup vs baseline: 1.0494x; 1.0494x over previous
"""Trainium2 Bass kernel for LowRankTriLinearFusionAttn.

Math (per sample b):
  g  = relu(LN(h_g  @ Wg.T + bg))          (256)
  d2 = relu(LN(h_2d @ W2.T + b2))          (256)
  d3 = relu(LN(h_3d @ W3.T + b3))          (256)
  z_r[b,r,:] = (g U_r^T) * (d2 V_r^T) * (d3 S_r^T)     r in 0..15
  beta = softmax(relu([h_g|h_2d|h_3d] @ Wa1.T + ba1) @ Wa2.T + ba2)
  z[b,:] = sum_r beta[b,r] * z_r[b,r,:]

Sharding: pure data parallel over 8 NeuronCores (batch 8192 -> 1024/core).
Host pre-packs weights transposed/bf16 AND the activations transposed
(features-major, bf16) so no x-transposes are needed on the PE.

Engine split in the rank phase (the 60%-of-kernel steady state):
  PE   : rank-expansion matmuls (48 x N=512 per b-tile)
  ACT  : beta-scaled PSUM evictions (per-partition scale = beta[:, r])
  DVE  : first trilinear multiply + bf16 accumulator adds (2x mode)
  GpSimd: second trilinear multiply
"""

import sys
import types

import numpy as np
import ml_dtypes

import concourse.bass as bass
import concourse.tile as tile
from concourse import bacc
from concourse import mybir
from concourse.bass import ts
from concourse.bass_utils import run_bass_kernel_spmd
import bass_rust


def _ensure_ntff_hook():
    """Provide antenv.axon_hooks if the image's antenv stub lacks it, so
    run_bass_kernel_spmd(trace=True) can capture NTFF profiles under axon."""
    try:
        import antenv.axon_hooks  # noqa: F401
        return
    except ImportError:
        pass
    try:
        from trn_agent_boot.trn_boot import _ntff_profile_via_ctypes

        hook = _ntff_profile_via_ctypes("/opt/axon/libaxon_pjrt.so")
    except Exception:
        hook = None
    mod = types.ModuleType("antenv.axon_hooks")
    _state = {"hook": hook}
    mod.get_axon_ntff_profile_hook = lambda: _state["hook"]
    mod.set_axon_ntff_profile_hook = lambda h: _state.update(hook=h)
    sys.modules["antenv.axon_hooks"] = mod


_ensure_ntff_hook()

BF16 = mybir.dt.bfloat16
F32 = mybir.dt.float32
AF = mybir.ActivationFunctionType
OP = mybir.AluOpType

N_CORES = 8
B = 8192
D_G, D_2D, D_3D = 512, 768, 1024
D_CAT = D_G + D_2D + D_3D  # 2304
D_F, RANK, ATTN_H = 256, 16, 512
RD = RANK * D_F  # 4096
P = 128

BC = B // N_CORES           # 1024 samples per core
NBT = BC // P               # 8 batch tiles per core
KD = [D_G // P, D_2D // P, D_3D // P]   # k-tiles per modality: 4, 6, 8
KOFF = [0, KD[0], KD[0] + KD[1]]        # xt chunk offsets: 0, 4, 10
NK = D_CAT // P             # 18
NH = ATTN_H // P            # 4
NCH = RD // 512             # 8 chunks of 512 in the rank-expanded dim
KF = D_F // P               # 2 k-tiles for the 256-dim contraction
EPS = 1e-5
N_WARM = 16                 # PE warm-up transposes before real work


def build_kernel(bc=BC, general_ln=False, use_bias=False, use_ba2=False):
    assert bc % 512 == 0
    nbt = bc // P
    nc = bacc.Bacc("TRN2", debug=False)

    # ---- external I/O (per-core shapes) ----
    xt_in = nc.dram_tensor("xt", [D_CAT, bc], BF16, kind="ExternalInput").ap()
    wc_t = nc.dram_tensor("wc_t", [D_CAT, D_F], BF16, kind="ExternalInput").ap()
    uvs_t = nc.dram_tensor("uvs_t", [6 * P, RD], BF16, kind="ExternalInput").ap()
    wa1_t = nc.dram_tensor("wa1_t", [D_CAT, ATTN_H], BF16, kind="ExternalInput").ap()
    wa2_t = nc.dram_tensor("wa2_t", [ATTN_H, RANK], BF16, kind="ExternalInput").ap()
    consts_f = nc.dram_tensor("consts_f", [P, 16], F32, kind="ExternalInput").ap()
    consts_b = nc.dram_tensor("consts_b", [4, D_F], BF16, kind="ExternalInput").ap()
    ident_in = nc.dram_tensor("ident", [P, P], BF16, kind="ExternalInput").ap()
    z_out = nc.dram_tensor("z", [bc, D_F], F32, kind="ExternalOutput").ap()

    from contextlib import ExitStack

    with tile.TileContext(nc) as tc, ExitStack() as ctx:
        consts = ctx.enter_context(tc.tile_pool(name="consts", bufs=1))
        wpool = ctx.enter_context(tc.tile_pool(name="w", bufs=1))
        xtp = ctx.enter_context(tc.tile_pool(name="xt", bufs=1))
        gtp = ctx.enter_context(tc.tile_pool(name="gt", bufs=1))
        sp = ctx.enter_context(tc.tile_pool(name="sp", bufs=16))
        zp = ctx.enter_context(tc.tile_pool(name="zacc", bufs=4))
        cp = ctx.enter_context(tc.tile_pool(name="cp", bufs=8))
        pp = ctx.enter_context(tc.tile_pool(name="ps", bufs=8, space="PSUM"))

        # ---------- constants / small weights (scalar HWDGE queue) ----------
        # wc per modality so the first projections gate on a small DMA
        wc_sb = wpool.tile([P, NK, D_F], BF16, tag="wc")
        wc_r = wc_t.rearrange("(t p) n -> p t n", p=P)
        for m in range(3):
            nc.scalar.dma_start(
                out=wc_sb[:, KOFF[m] : KOFF[m] + KD[m], :],
                in_=wc_r[:, KOFF[m] : KOFF[m] + KD[m], :],
            )
        cf_sb = consts.tile([P, 16], F32, tag="cf")  # ba1(4) lnw(6) lnb(6)
        nc.scalar.dma_start(out=cf_sb, in_=consts_f)
        identity = consts.tile([P, P], BF16, tag="ident")
        nc.scalar.dma_start(out=identity, in_=ident_in)
        ba1_sb = cf_sb[:, 0:NH]
        lnw_sb = cf_sb[:, 4:10]
        lnb_sb = cf_sb[:, 10:16]
        if use_bias or use_ba2:
            cb_sb = consts.tile([1, 4, D_F], BF16, tag="cb")
            nc.scalar.dma_start(
                out=cb_sb, in_=consts_b.rearrange("(o m) n -> o m n", o=1)
            )
            bias_sb = cb_sb[:, 0:3, :]
            ba2_sb = cb_sb[:, 3, 0:RANK]
            ones_row = consts.tile([1, P], BF16, tag="ones")
            nc.vector.memset(ones_row, 1.0)
        eps_t = consts.tile([P, 1], F32, tag="eps")
        nc.vector.memset(eps_t, EPS)
        warm_sb = consts.tile([P, P], BF16, tag="warm")
        nc.vector.memset(warm_sb, 1.0)

        # ---------- transposed activations (sync HWDGE queue, k-order) ----
        # grouped DMAs amortize the ~0.64us per-DMA issue cost; group edges
        # align with the modality boundaries the m-major proj sweep needs
        xt_sb = xtp.tile([P, NK, bc], BF16, tag="xt")
        xt = [xt_sb[:, k, :] for k in range(NK)]
        xt_r = xt_in.rearrange("(t p) n -> p t n", p=P)
        xt_dmas = []
        for a, b in ((0, 2), (2, 4), (4, 7), (7, 10), (10, 14), (14, 18)):
            d = nc.sync.dma_start(out=xt_sb[:, a:b, :], in_=xt_r[:, a:b, :])
            xt_dmas.append(d)

        # ---------- big weights ----------
        # All big-weight DMAs ride the gpsimd queue, each gated on the
        # PREVIOUS transfer's completion via a tiny probe read (a dep between
        # two DMA instructions only orders their issue, not the transfers):
        # xt stream finishes at full HBM bandwidth, then wa1, then uvs.
        probe = consts.tile([1, 2], BF16, tag="probe")
        nc.gpsimd.tensor_copy(probe, xt[9][0:1, 0:2])
        wa1_sb = wpool.tile([P, NK, ATTN_H], BF16, tag="wa1")
        wa1_r = wa1_t.rearrange("(t p) n -> p t n", p=P)
        d_wa1 = nc.gpsimd.dma_start(out=wa1_sb, in_=wa1_r)
        wa2_sb = wpool.tile([P, NH, RANK], BF16, tag="wa2")
        d_wa2 = nc.gpsimd.dma_start(
            out=wa2_sb, in_=wa2_t.rearrange("(t p) n -> p t n", p=P)
        )
        probe2 = consts.tile([1, 2], BF16, tag="probe2")
        nc.gpsimd.tensor_copy(probe2, wa1_sb[0:1, 0, 0:2])
        uvs_sb = wpool.tile([P, 6, RD], BF16, tag="uvs")
        uvs_r = uvs_t.rearrange("(t p) n -> p t n", p=P)
        d_uvs = []
        for q in range(4):
            d = nc.gpsimd.dma_start(
                out=uvs_sb[:, :, ts(q, RD // 4)], in_=uvs_r[:, :, ts(q, RD // 4)]
            )
            d_uvs.append(d)

        # ---------- PE warm-up: ramp the clock while DMAs land ----------
        for w in range(N_WARM):
            wps = pp.tile([P, P], BF16, tag="ps", name="warm")
            nc.tensor.transpose(wps, warm_sb, warm_sb)

        # ---------- projections + LN (m-major follows DMA arrival) -------
        ups = [[None] * 3 for _ in range(nbt)]

        def emit_proj(t, m):
            ps = pp.tile([P, D_F], F32, tag="ps", name="ps_proj")
            for k in range(KD[m]):
                nc.tensor.matmul(
                    ps,
                    lhsT=xt[KOFF[m] + k][:, ts(t, P)],
                    rhs=wc_sb[:, KOFF[m] + k, :],
                    start=(k == 0),
                    stop=(k == KD[m] - 1) and not use_bias,
                )
            if use_bias:
                nc.tensor.matmul(
                    ps, lhsT=ones_row, rhs=bias_sb[:, m, :], start=False, stop=True
                )
            stats = sp.tile([P, 6], F32, tag="stats", name="stats")
            nc.vector.bn_stats(stats, ps)
            mv = sp.tile([P, 2], F32, tag="mv", name="mv")
            nc.vector.bn_aggr(mv, stats)
            sd = sp.tile([P, 1], F32, tag="sd", name="sd")
            nc.scalar.activation(sd, mv[:, 1:2], AF.Sqrt, bias=eps_t, scale=1.0)
            rstd = sp.tile([P, 1], F32, tag="rstd", name="rstd")
            nc.vector.reciprocal(rstd, sd)
            u = gtp.tile([P, D_F], BF16, tag=f"u{t}_{m}", name=f"u{t}_{m}")
            nc.vector.tensor_scalar(
                out=u,
                in0=ps,
                scalar1=mv[:, 0:1],
                scalar2=rstd,
                op0=OP.subtract,
                op1=OP.mult,
            )
            ups[t][m] = u

        for m in range(3):
            for t in range(nbt):
                emit_proj(t, m)

        # ---------- attention layer 1 (transposed: h on partitions) ------
        a1t = wpool.tile([P, NH, bc], BF16, tag="a1t")

        def emit_attn_l1(c, h):
            ps = pp.tile([P, 512], F32, tag="ps", name="ps_a1")
            for k in range(NK):
                nc.tensor.matmul(
                    ps,
                    lhsT=wa1_sb[:, k, ts(h, P)],
                    rhs=xt[k][:, ts(c, 512)],
                    start=(k == 0),
                    stop=(k == NK - 1),
                )
            nc.scalar.activation(
                a1t[:, h, ts(c, 512)],
                ps,
                AF.Relu,
                bias=ba1_sb[:, h : h + 1],
                scale=1.0,
            )

        # ---------- per-tile tail stages ----------
        gts = [None] * nbt
        betas = [None] * nbt

        def emit_a2_softmax(t):
            ps = pp.tile([P, RANK], F32, tag="ps", name="ps_a2")
            for k in range(NH):
                nc.tensor.matmul(
                    ps,
                    lhsT=a1t[:, k, ts(t, P)],
                    rhs=wa2_sb[:, k, :],
                    start=(k == 0),
                    stop=(k == NH - 1) and not use_ba2,
                )
            if use_ba2:
                nc.tensor.matmul(ps, lhsT=ones_row, rhs=ba2_sb, start=False, stop=True)
            negm = sp.tile([P, 1], F32, tag="negm", name="negm")
            nc.vector.reduce_max(negm, ps, axis=mybir.AxisListType.X, negate=True)
            e = sp.tile([P, RANK], F32, tag="esm", name="esm")
            ssum = sp.tile([P, 1], F32, tag="ssum", name="ssum")
            nc.scalar.activation(e, ps, AF.Exp, bias=negm, scale=1.0, accum_out=ssum)
            rs = sp.tile([P, 1], F32, tag="rs", name="rs")
            nc.vector.reciprocal(rs, ssum)
            beta = gtp.tile([P, RANK], F32, tag=f"beta{t}", name=f"beta{t}")
            nc.vector.tensor_scalar_mul(beta, e, rs)
            betas[t] = beta

        def emit_gtt(t):
            gt = gtp.tile([P, 6, P], BF16, tag=f"gt{t}", name=f"gt{t}")
            for m in range(3):
                u = ups[t][m]
                tp = pp.tile([P, KF, P], BF16, tag="ps", name="tpg")
                for j in range(KF):
                    nc.tensor.transpose(tp[:, j, :], u[:, ts(j, P)], identity)
                if general_ln:
                    for j in range(KF):
                        col = m * KF + j
                        nc.scalar.activation(
                            gt[:, m * KF + j, :],
                            tp[:, j, :],
                            AF.Relu,
                            bias=lnb_sb[:, col : col + 1],
                            scale=lnw_sb[:, col : col + 1],
                        )
                else:
                    # identity LN affine: relu-only evict (ACT; DVE is the
                    # critical engine in the rank waves)
                    nc.scalar.activation(
                        gt[:, m * KF : (m + 1) * KF, :], tp, AF.Relu,
                        bias=0.0, scale=1.0,
                    )
            gts[t] = gt

        def emit_rank(t):
            beta = betas[t]
            gt = gts[t]
            acc = zp.tile([P, 512], BF16, tag="acc", name="acc")
            for c in range(NCH):
                pz = []
                for m in range(3):
                    ps = pp.tile([P, 512], F32, tag="ps", name="ps_rk")
                    for k in range(KF):
                        nc.tensor.matmul(
                            ps,
                            lhsT=gt[:, m * KF + k, :],
                            rhs=uvs_sb[:, m * KF + k, ts(c, 512)],
                            start=(k == 0),
                            stop=(k == KF - 1),
                        )
                    pz.append(ps)
                # beta folded into the pz0 eviction (per-partition scale, ACT)
                ugb = cp.tile([P, 512], BF16, tag="ugb", name="ugb")
                for rr in range(2):
                    r = 2 * c + rr
                    nc.scalar.activation(
                        ugb[:, ts(rr, D_F)],
                        pz[0][:, ts(rr, D_F)],
                        AF.Copy,
                        scale=beta[:, r : r + 1],
                    )
                tm = cp.tile([P, 512], BF16, tag="tm", name="tm")
                nc.vector.tensor_tensor(tm, ugb, pz[1], op=OP.mult)
                if c == 0:
                    nc.vector.tensor_tensor(acc, tm, pz[2], op=OP.mult)
                else:
                    t2 = cp.tile([P, 512], BF16, tag="t2", name="t2")
                    nc.vector.tensor_tensor(t2, tm, pz[2], op=OP.mult)
                    # bf16 SBUF-only accumulate rides the otherwise-idle GpSimd
                    nc.gpsimd.tensor_tensor(acc, t2, acc, op=OP.add)
            zfin = zp.tile([P, D_F], F32, tag="zfin", name="zfin")
            nc.gpsimd.tensor_tensor(
                zfin, acc[:, 0:D_F], acc[:, D_F : 2 * D_F], op=OP.add
            )
            nc.sync.dma_start(out=z_out[ts(t, P), :], in_=zfin)

        # ---------- wave schedule ----------
        # gtt transposes interleave with attn1 h-groups (each gtt only needs
        # its own LN chain done, not the whole DVE backlog). attn1 chunk-1
        # h-groups are spread between early rank tiles: each is ~4us of PE
        # work with no ACT/DVE load, a catch-up window for the trilinear
        # elementwise backlog; late gtt evicts ride those windows too.
        emit_attn_l1(0, 0)
        emit_gtt(0)
        emit_attn_l1(0, 1)
        emit_gtt(1)
        emit_attn_l1(0, 2)
        emit_gtt(2)
        emit_attn_l1(0, 3)
        for t in range(4):
            emit_a2_softmax(t)
        emit_rank(0)
        emit_attn_l1(1, 0)
        emit_gtt(3)
        emit_rank(1)
        emit_attn_l1(1, 1)
        emit_gtt(4)
        emit_rank(2)
        emit_attn_l1(1, 2)
        emit_gtt(5)
        emit_rank(3)
        emit_attn_l1(1, 3)
        emit_gtt(6)
        emit_gtt(7)
        for t in range(4, nbt):
            emit_a2_softmax(t)
        for t in range(4, nbt):
            emit_rank(t)

    nc.compile()
    return nc


_BF = ml_dtypes.bfloat16


def _pack_weights(inputs):
    """Host-side offline packing: transpose + cast weights once."""
    f = np.asarray
    wc_t = np.concatenate(
        [f(inputs["Wg"]).T, f(inputs["W2"]).T, f(inputs["W3"]).T], axis=0
    ).astype(_BF)  # [2304, 256]
    uvs_t = np.concatenate(
        [f(inputs["U"]).T, f(inputs["V"]).T, f(inputs["S"]).T], axis=0
    ).astype(_BF)  # [768, 4096]
    wa1_t = np.ascontiguousarray(f(inputs["Wa1"]).T).astype(_BF)  # [2304, 512]
    wa2_t = np.ascontiguousarray(f(inputs["Wa2"]).T).astype(_BF)  # [512, 16]
    consts_b = np.zeros((4, D_F), dtype=_BF)
    consts_b[0] = f(inputs["bg"]).astype(_BF)
    consts_b[1] = f(inputs["b2"]).astype(_BF)
    consts_b[2] = f(inputs["b3"]).astype(_BF)
    consts_b[3, :RANK] = f(inputs["ba2"]).astype(_BF)
    consts_f = np.concatenate(
        [
            f(inputs["ba1"]).reshape(NH, P).T,
            np.concatenate(
                [
                    f(inputs["ln_g_w"]).reshape(KF, P),
                    f(inputs["ln_2_w"]).reshape(KF, P),
                    f(inputs["ln_3_w"]).reshape(KF, P),
                ],
                axis=0,
            ).T,
            np.concatenate(
                [
                    f(inputs["ln_g_b"]).reshape(KF, P),
                    f(inputs["ln_2_b"]).reshape(KF, P),
                    f(inputs["ln_3_b"]).reshape(KF, P),
                ],
                axis=0,
            ).T,
        ],
        axis=1,
    ).astype(np.float32)  # [128, 16]
    return {
        "ident": np.eye(P, dtype=_BF),
        "wc_t": wc_t,
        "uvs_t": uvs_t,
        "wa1_t": wa1_t,
        "wa2_t": wa2_t,
        "consts_f": consts_f,
        "consts_b": consts_b,
    }


_NC_CACHE = {}


def _get_nc(general_ln, use_bias, use_ba2):
    key = (general_ln, use_bias, use_ba2)
    if key not in _NC_CACHE:
        _NC_CACHE[key] = build_kernel(
            general_ln=general_ln, use_bias=use_bias, use_ba2=use_ba2
        )
    return _NC_CACHE[key]


def kernel(run_opts=None, **inputs):
    f64 = np.float64
    general_ln = not (
        np.all(np.asarray(inputs["ln_g_w"], f64) == 1.0)
        and np.all(np.asarray(inputs["ln_2_w"], f64) == 1.0)
        and np.all(np.asarray(inputs["ln_3_w"], f64) == 1.0)
        and np.all(np.asarray(inputs["ln_g_b"], f64) == 0.0)
        and np.all(np.asarray(inputs["ln_2_b"], f64) == 0.0)
        and np.all(np.asarray(inputs["ln_3_b"], f64) == 0.0)
    )
    use_bias = not (
        np.all(np.asarray(inputs["bg"], f64) == 0.0)
        and np.all(np.asarray(inputs["b2"], f64) == 0.0)
        and np.all(np.asarray(inputs["b3"], f64) == 0.0)
    )
    use_ba2 = not np.all(np.asarray(inputs["ba2"], f64) == 0.0)
    nc = _get_nc(general_ln, use_bias, use_ba2)
    wmap = _pack_weights(inputs)
    x_all = np.concatenate(
        [
            np.asarray(inputs["h_g"], dtype=np.float32),
            np.asarray(inputs["h_2d"], dtype=np.float32),
            np.asarray(inputs["h_3d"], dtype=np.float32),
        ],
        axis=1,
    )  # [B, 2304]

    in_maps = []
    for i in range(N_CORES):
        sl = slice(i * BC, (i + 1) * BC)
        m = dict(wmap)
        m["xt"] = x_all[sl].T.astype(_BF)  # [2304, 1024] contiguous bf16
        in_maps.append(m)

    res = run_bass_kernel_spmd(
        nc, in_maps, core_ids=list(range(N_CORES)), **(run_opts or {})
    )
    out = np.concatenate([r["z"] for r in res.results], axis=0)
    if run_opts:
        kernel.last_results = res
    return out
